# revision 3
# baseline (speedup 1.0000x reference)
"""Trainium2 Bass kernel for CascadedPathEncoder.

Reference computation (per sample b):
    h_0 = relu(W_0 @ [0_256; wp_0] + b_0)
    h_p = relu(W_p @ [h_{p-1}; wp_p] + b_p)      p = 1..31
    out[b] = concat_p h_p                         -> [8192, 8192]

Strategy: pure data parallel over 8 NeuronCores (1024 batch rows each),
bf16 compute with f32 PSUM accumulation. Per core the hidden state
lives transposed in SBUF as one bf16 [128, 2, 2, 512] tile per step
(partition = hidden-within-chunk, dims = m-chunk, batch-tile, batch).

Per step each of the 4 PSUM banks (m-chunk x batch-tile) accumulates
a K=4 wp matmul (opens the group) plus two K=128 chunks of the
previous hidden state. The wp matmuls of one step are issued as ONE
span of four concurrent 32x128 row-tiled matmuls: the 4 wp rows and
wp weights are replicated into all four 32-partition quadrants (one
combined wpx tensor), so tile q = (m,t) reads partitions 32q..32q+3
and writes its own PSUM bank. The span costs ~1 matmul slot instead
of 4 serial full-array matmuls with 97%-zero lhsT.

The h matmuls run t-outer, (k0,m0),(k1,m0),(k0,m1),(k1,m1) within
each batch-tile, so banks close at slots 2/4/6/8 of the h-chain.
Bias+relu interleave: ACT takes the m0 banks (slots 2, 6), DVE the m1
banks (slots 4, 8); next-step k0 matmuls then wait only on an ACT
product and k1 only on a DVE product, hiding the relu ring under the
PE conveyor. ~NWARM warmup matmuls on a memset tile run while inputs
load, so the HAM clock gate (PE at 1.2 GHz until ~3.4us of sustained
activity) flips before the first real matmul. Outputs stream per step
as two 256KB DMAs on the sync HWDGE ring; wh streams on the GpSimd
SWDGE ring so outputs never queue behind the 4 MiB weight stream.
The last two steps split 3 ways (+scalar ring) to shorten the drain
tail.

Host re-assembles the full [8192, 8192] f32 from bf16 step outputs.
"""

import numpy as np
import ml_dtypes

BF16 = ml_dtypes.bfloat16

P = 32          # scan steps
PD = 4          # point dim
H = 256         # hidden dim
B = 8192        # global batch
NCORES = 8
BS = B // NCORES  # 1024 rows per core
TN = 512        # matmul moving free dim (one PSUM bank of f32; ISA max)
NT = BS // TN   # batch tiles per core
NWARM = 66      # PE warmup matmuls (HAM un-throttle before first real MM)
WP_SPAN = True  # row-tiled concurrent wp span (False: baseline K=128 wp)

_CACHE = {}


def _build_nc():
    from contextlib import ExitStack

    import concourse.bass as bass
    import concourse.tile as tile
    from concourse import bacc, mybir

    dt = mybir.dt
    ts = bass.ts

    nc = bacc.Bacc(
        "TRN2", target_bir_lowering=False, debug=False, num_devices=NCORES
    )
    # wh[kk, p, k, jj] = W[p, jj + 128m, 128k + kk] (lhsT for the h chunks)
    wh = nc.dram_tensor("wh", [128, P, 2, 256], dt.bfloat16, kind="ExternalInput").ap()
    # Quadrant-replicated wp data: quadrant q serves (m, t) = (q>>1, q&1).
    # wpx[4q + r, p, 0:512]     = path_data[c*BS + (q&1)*512 + b, 4p + r]
    # wpx[4q + r, p, 512 + j]   = W[p, 128*(q>>1) + j, 256 + r]
    wpx = nc.dram_tensor("wpx", [16, P, TN + 128], dt.bfloat16, kind="ExternalInput").ap()
    bias = nc.dram_tensor("bias", [128, P, 2], dt.float32, kind="ExternalInput").ap()
    if not WP_SPAN:
        wx = nc.dram_tensor("wx", [128, P, 2, 128], dt.bfloat16, kind="ExternalInput").ap()
        pdx = nc.dram_tensor("pdx", [128, BS], dt.bfloat16, kind="ExternalInput").ap()
    out = nc.dram_tensor(
        "out", [P, 128, 2, NT, TN], dt.bfloat16, kind="ExternalOutput"
    ).ap()

    with tile.TileContext(nc) as tc, ExitStack() as ctx:
        const = ctx.enter_context(tc.tile_pool(name="const", bufs=1))
        state = ctx.enter_context(tc.tile_pool(name="state", bufs=10))
        psum = ctx.enter_context(tc.tile_pool(name="psum", bufs=2, space="PSUM"))

        wpx_sb = const.tile([128, P, TN + 128], dt.bfloat16)
        b_sb = const.tile([128, P, 2], dt.float32)
        wh_sb = const.tile([128, P, 2, 256], dt.bfloat16)
        warm_sb = const.tile([128, 64], dt.bfloat16)
        if not WP_SPAN:
            wx_sb = const.tile([128, P, 2, 128], dt.bfloat16)
            pdx_sb = const.tile([128, BS], dt.bfloat16)

        # Input DMAs. Start-gating tensors (bias, wpx quadrants) ride the
        # sync ring head; the 4 MiB wh stream rides the GpSimd SWDGE ring
        # so step outputs (sync ring, from step 0 on) never queue behind
        # it. Each dma_start dispatch occupies its engine ~0.6-1.2us, and
        # a consumer waits on its WHOLE transfer's completion semaphore.
        nc.sync.dma_start(out=b_sb[:], in_=bias[:])
        for q in range(4):
            nc.sync.dma_start(
                out=wpx_sb[32 * q : 32 * q + 4, :, :], in_=wpx[4 * q : 4 * q + 4, :, :]
            )
        if not WP_SPAN:
            nc.scalar.dma_start(out=pdx_sb[:], in_=pdx[:])
            nc.sync.dma_start(out=wx_sb[:], in_=wx[:])
        nc.gpsimd.dma_start(out=wh_sb[:, 0:2, :, :], in_=wh[:, 0:2, :, :])
        nc.gpsimd.dma_start(out=wh_sb[:, 2:4, :, :], in_=wh[:, 2:4, :, :])
        nc.gpsimd.dma_start(out=wh_sb[:, 4:8, :, :], in_=wh[:, 4:8, :, :])
        nc.gpsimd.dma_start(out=wh_sb[:, 8:16, :, :], in_=wh[:, 8:16, :, :])
        nc.gpsimd.dma_start(out=wh_sb[:, 16:24, :, :], in_=wh[:, 16:24, :, :])
        nc.gpsimd.dma_start(out=wh_sb[:, 24:32, :, :], in_=wh[:, 24:32, :, :])

        # PE warmup: flip the HAM clock gate to 8/8 while inputs stream.
        nc.vector.memset(warm_sb[:], 0.0)
        # ACT table preload: the first ACTIVATE pays a ~1.3us Relu
        # ACT_TABLE_LOAD; trigger it on scratch during the input wait.
        warm_out = const.tile([128, 8], dt.bfloat16)
        nc.scalar.activation(
            warm_out[:],
            warm_sb[:, 0:8],
            mybir.ActivationFunctionType.Relu,
            scale=1.0,
        )
        warm_ps = psum.tile([128, NT, TN], dt.float32, tag="ps_m0", name="warm")
        for i in range(NWARM):
            nc.tensor.matmul(
                warm_ps[0:64, 0, 0:64],
                lhsT=warm_sb[:],
                rhs=warm_sb[:],
                start=True,
                stop=True,
                skip_group_check=True,
            )

        h_prev = None
        for p in range(P):
            ps = [
                psum.tile(
                    [128, NT, TN],
                    dt.float32,
                    tag=f"ps_m{m}",
                    name=f"ps_p{p}m{m}",
                )
                for m in range(2)
            ]
            # wp pass opens each accumulation group. One span of 4
            # concurrent row-tiled K=4 matmuls; quadrant q -> (m, t).
            # Banks were freed by step p-2's relus, so at execution time
            # (right after step p-1's h chain) nothing stalls.
            if WP_SPAN:
                for q in range(4):
                    m, t = q >> 1, q & 1
                    nc.tensor.matmul(
                        ps[m][:, t, :],
                        lhsT=wpx_sb[32 * q : 32 * q + 4, p, TN : TN + 128],
                        rhs=wpx_sb[32 * q : 32 * q + 4, p, 0:TN],
                        start=True,
                        stop=(p == 0),
                        tile_position=(32 * q, 0),
                    )
            else:
                for m in range(2):
                    for t in range(NT):
                        nc.tensor.matmul(
                            ps[m][:, t, :],
                            lhsT=wx_sb[:, p, m, :],
                            rhs=pdx_sb[:, ts(t, TN)],
                            start=True,
                            stop=(p == 0),
                        )
            hn = state.tile(
                [128, 2, NT, TN], dt.bfloat16, tag="h", name=f"h_p{p}"
            )

            def relu_act(t):
                nc.scalar.activation(
                    hn[:, 0, t, :],
                    ps[0][:, t, :],
                    mybir.ActivationFunctionType.Relu,
                    bias=b_sb[:, p, 0:1],
                    scale=1.0,
                )

            def relu_dve(t):
                nc.vector.tensor_scalar(
                    hn[:, 1, t, :],
                    ps[1][:, t, :],
                    scalar1=b_sb[:, p, 1:2],
                    scalar2=0.0,
                    op0=mybir.AluOpType.add,
                    op1=mybir.AluOpType.max,
                )

            if p > 0:
                # t-outer; per t: (k0,m0),(k1,m0),(k0,m1),(k1,m1) so banks
                # close at slots 2/4/6/8; relu fires as each bank closes.
                # k0 matmuls consume ACT products, k1 consume DVE products.
                # Last two steps close the t=1 banks first so their relu
                # products ship ~1.3us earlier, shortening the drain.
                t_order = range(NT) if p < P - 2 else range(NT - 1, -1, -1)
                for t in t_order:
                    for m in range(2):
                        for k in range(2):
                            nc.tensor.matmul(
                                ps[m][:, t, :],
                                lhsT=wh_sb[:, p, k, ts(m, 128)],
                                rhs=h_prev[:, k, t, :],
                                start=False,
                                stop=(k == 1),
                            )
                        if m == 0:
                            relu_act(t)
                        else:
                            relu_dve(t)
            else:
                for t in range(NT):
                    relu_act(t)
                    relu_dve(t)
            # outputs: sync ring is output-only from step 0 (wh rides the
            # SWDGE ring), so these never lag behind the input stream.
            if p >= P - 2:
                # tail: t=1 products (computed first) ship immediately,
                # each on its own ring
                nc.scalar.dma_start(out=out[p, :, 0, 1, :], in_=hn[:, 0, 1, :])
                nc.sync.dma_start(out=out[p, :, 1, 1, :], in_=hn[:, 1, 1, :])
                nc.sync.dma_start(out=out[p, :, :, 0, :], in_=hn[:, :, 0, :])
            else:
                nc.sync.dma_start(out=out[p, :, :, 0, :], in_=hn[:, :, 0, :])
                nc.sync.dma_start(out=out[p, :, :, 1, :], in_=hn[:, :, 1, :])
            h_prev = hn

    nc.compile()
    return nc


def _get_nc():
    if "nc" not in _CACHE:
        _CACHE["nc"] = _build_nc()
    return _CACHE["nc"]


def _pack_inputs(path_data, W, b):
    """Host-side packing into the DRAM layouts the kernel expects."""
    # lhsT for the two K=128 chunks: wh[kk, p, k, jj] = W[p, jj, 128k+kk]
    wh_np = np.ascontiguousarray(
        W[:, :, :H].reshape(P, H, 2, 128).transpose(3, 0, 2, 1)
    ).astype(BF16)
    # Quadrant-replicated wp block: quadrant q serves (m, t) = (q>>1, q&1)
    # wpx[4q+r, p, 0:512]   = path_data[c*BS + (q&1)*512 + b, 4p+r]
    # wpx[4q+r, p, 512+j]   = W[p, 128*(q>>1)+j, 256+r]
    # wxs[r, p, m, j] = W[p, 128m+j, 256+r]
    wxs = W[:, :, H:].reshape(P, 2, 128, PD).transpose(3, 0, 1, 2).astype(BF16)
    # pds[c][r, p, t, b] = path_data[c*BS + t*512 + b, 4p+r]
    pds = [
        np.ascontiguousarray(
            path_data[c * BS : (c + 1) * BS]
            .reshape(NT, TN, P, PD)
            .transpose(3, 2, 0, 1)
        ).astype(BF16)
        for c in range(NCORES)
    ]
    wpx_all = []
    for c in range(NCORES):
        wpx_np = np.empty((16, P, TN + 128), dtype=BF16)
        for q in range(4):
            m, t = q >> 1, q & 1
            wpx_np[4 * q : 4 * q + 4, :, :TN] = pds[c][:, :, t, :]
            wpx_np[4 * q : 4 * q + 4, :, TN:] = wxs[:, :, m, :]
        wpx_all.append(wpx_np)
    # bias[j, p, m] = b[p, 128m+j]
    b_np = np.ascontiguousarray(b.reshape(P, 2, 128).transpose(2, 0, 1)).astype(
        np.float32
    )
    return wh_np, wpx_all, b_np


def _make_in_maps(path_data, W, b):
    wh_np, wpx_all, b_np = _pack_inputs(path_data, W, b)
    maps = [
        {"wh": wh_np, "wpx": wpx_all[c], "bias": b_np}
        for c in range(NCORES)
    ]
    if not WP_SPAN:
        wx_np = np.zeros((128, P, 2, 128), dtype=BF16)
        wxs = W[:, :, H:].reshape(P, 2, 128, PD).transpose(3, 0, 1, 2).astype(BF16)
        for p in range(P):
            wx_np[4 * p : 4 * p + 4, p] = wxs[:, p]
        for c in range(NCORES):
            maps[c]["wx"] = wx_np
            maps[c]["pdx"] = np.ascontiguousarray(
                path_data[c * BS : (c + 1) * BS].T
            ).astype(BF16)
    return maps


def _unpack_out(results):
    # out[p, jj, m, t, bb] -> full[c*BS + t*TN + bb, p*256 + m*128 + jj]
    return np.concatenate(
        [
            np.asarray(r["out"])
            .transpose(3, 4, 0, 2, 1)
            .reshape(BS, P * H)
            .astype(np.float32)
            for r in results
        ],
        axis=0,
    )


def kernel(path_data, W, b):
    from concourse.bass_utils import run_bass_kernel_spmd

    path_data = np.asarray(path_data, dtype=np.float32)
    W = np.asarray(W, dtype=np.float32)
    b = np.asarray(b, dtype=np.float32)

    in_maps = _make_in_maps(path_data, W, b)
    nc = _get_nc()
    res = run_bass_kernel_spmd(nc, in_maps, core_ids=list(range(NCORES)))
    return _unpack_out(res.results)
